# revision 12
# baseline (speedup 1.0000x reference)
"""Multi-head attention (N=2, S=4096, E=512, H=8) on 8 TRN2 NeuronCores.

Sharding: sequence-parallel over (batch, q-chunk): core c handles batch
c//4, query rows (c%4)*1024 .. +1024, computing all 8 heads for those
queries locally (K/V are computed for the full 4096-key sequence of its
batch).  No collectives; the host concatenates the 8 output slices.

v2 changes vs v1 (641us):
  - exp is split between ScalarE (exact, table exp) and DVE (Schraudolph
    int16-bitcast-to-bf16 via one tensor_scalar), alternating per key
    tile, so neither engine paces the PE attention stream.
  - finer attention pipeline: one 128-key tile per psum group
    ([128, 2heads, 512q] psum, rotation 2; O matmuls lag by 2 groups) to
    keep PE gap-free (PE only reaches its 2.4 GHz pstate after ~3us of
    continuous execution; v1 ran most matmuls at the 1.2 GHz mid state).
  - softmax denominators: reciprocal_approx_fast (1 DVE pass) instead of
    full-precision reciprocal (6 passes, 52us of DVE in v1); the
    broadcast and the normalize multiply run on GpSimd.
  - projection psum->SBUF copies are spread across DVE and ScalarE.
"""

import math
import sys

for _p in ("/opt/trn_rl_repo", "/opt/pypackages"):
    if _p not in sys.path:
        sys.path.append(_p)

import numpy as np

import concourse.bass as bass  # noqa: F401
import concourse.mybir as mybir
import concourse.tile as tile
from concourse import bacc
from concourse.bass_utils import run_bass_kernel_spmd

P = 128
N_BATCH = 2
S = 4096
E = 512
H = 8
HD = 64
NCORES = 8
QS = 1024  # query rows per core
KTT = S // P  # 32 key tiles
SCALE = 1.0 / math.sqrt(E)
# Schraudolph exp in bf16-bit domain: bits_i16 = round(s*A + B) viewed as
# bf16 ~= exp(s*SCALE).  B bias -6 minimizes max rel err (~3.5%) on the
# observed score range |s*SCALE| <= ~2.5.
A_SCH = SCALE * 128.0 / math.log(2.0)
B_SCH = 127.0 * 128.0 - 6.0
F32 = mybir.dt.float32
BF16 = mybir.dt.bfloat16
I16 = mybir.dt.int16
Exp = mybir.ActivationFunctionType.Exp
Mult = mybir.AluOpType.mult
Add = mybir.AluOpType.add

OLAG = 3  # O matmuls run this many key tiles behind S/exp


def build_core_program():
    nc = bacc.Bacc("TRN2", target_bir_lowering=False, debug=False)
    nc.num_devices = NCORES

    xq = nc.dram_tensor("xq", [QS, E], F32, kind="ExternalInput").ap()
    xk = nc.dram_tensor("xk", [QS, E], F32, kind="ExternalInput").ap()
    xv = nc.dram_tensor("xv", [QS, E], F32, kind="ExternalInput").ap()
    kslice_d = nc.dram_tensor("kslice_d", [P, 4, QS], BF16, kind="Internal").ap()
    kgath_d = nc.dram_tensor("kgath_d", [4, P, 4, QS], BF16, kind="Internal").ap()
    vslice_d = nc.dram_tensor("vslice_d", [P, 8, H * (HD + 1)], BF16, kind="Internal").ap()
    vgath_d = nc.dram_tensor(
        "vgath_d", [4, P, 8, H * (HD + 1)], BF16, kind="Internal"
    ).ap()
    w_in = {
        name: nc.dram_tensor(f"w{name}", [E, E], F32, kind="ExternalInput").ap()
        for name in ("q", "k", "v", "fc")
    }
    out = nc.dram_tensor("out", [QS, E], F32, kind="ExternalOutput").ap()

    from contextlib import ExitStack

    with tile.TileContext(nc) as tc, ExitStack() as ctx:
        ep = ctx.enter_context
        ci = ep(tc.tile_pool(name="ci", bufs=8))
        co = ep(tc.tile_pool(name="co", bufs=3))
        big = ep(tc.tile_pool(name="big", bufs=1))
        xch = ep(tc.tile_pool(name="xch", bufs=3))
        atp = ep(tc.tile_pool(name="atp", bufs=4))
        small = ep(tc.tile_pool(name="small", bufs=2))
        pp = ep(tc.tile_pool(name="pp", bufs=2, space="PSUM"))  # S/proj/fc psums
        ppo = ep(tc.tile_pool(name="ppo", bufs=3, space="PSUM"))  # O accumulators

        from concourse.masks import make_identity

        ident = big.tile([P, P], F32, tag="ident")
        make_identity(nc, ident[:])

        # rotate psum->sbuf copies between DVE and ScalarE
        _cp = [0]

        def copy_cast(dst, src):
            if _cp[0] % 2 == 0:
                nc.vector.tensor_copy(dst, src)
            else:
                nc.scalar.copy(dst, src)
            _cp[0] += 1

        # ---- staged transpose: one 512-row chunk of a fp32 [rows, E] input ->
        # SBUF [128, 4, 512] bf16 via PE transpose (features onto partitions) ----
        def stage_chunk(src, r0):
            tfs = []
            for rt in range(4):
                tf = ci.tile([P, E], F32, tag="ci", name="tf")
                nc.sync.dma_start(tf[:], src[r0 + rt * P : r0 + (rt + 1) * P, :])
                tfs.append(tf)
            xt = xch.tile([P, 4, 512], BF16, tag="xc", name="xt")
            for sub in range(4):
                ps = pp.tile([P, 512], F32, tag="s", name="pst")
                for rt in range(4):
                    nc.tensor.transpose(
                        ps[:, rt * P : (rt + 1) * P],
                        tfs[rt][:, sub * P : (sub + 1) * P],
                        ident[:],
                    )
                copy_cast(xt[:, sub, :], ps[:])
            return xt

        # weights: resident transposed copies
        wT = {}

        def stage_weights():
            for name in w_in:
                xt = stage_chunk(w_in[name], 0)
                wt = big.tile([P, 4, E], BF16, tag=f"w{name}", name="wt")
                nc.vector.tensor_copy(wt[:], xt[:])
                wT[name] = wt

        qT = big.tile([P, 4, QS], BF16, tag="qT")
        kT = big.tile([P, 4, S], BF16, tag="kT")
        Vp = big.tile([P, S // P, H * (HD + 1)], BF16, tag="Vp")
        GROUPS = [[0, 1, 2, 3], [4, 5, 6, 7]]

        # --- K: project own 1024-key quarter, allgather across the 4 cores
        # of this batch.  The gather also routes our own slice back, so the
        # program stays rank-agnostic. ---
        def stage_k():
            kOwn = big.tile([P, 4, QS], BF16, tag="kOwn")
            for kc in range(QS // 512):
                xt = stage_chunk(xk, kc * 512)
                for p4 in range(4):
                    ps = pp.tile([P, 512], F32, tag="s", name="psk")
                    for sub in range(4):
                        nc.tensor.matmul(
                            ps[:],
                            lhsT=wT["k"][:, sub, p4 * P : (p4 + 1) * P],
                            rhs=xt[:, sub, :],
                            start=(sub == 0),
                            stop=(sub == 3),
                        )
                    copy_cast(kOwn[:, p4, kc * 512 : (kc + 1) * 512], ps[:])
            nc.sync.dma_start(kslice_d[:], kOwn[:])
            nc.gpsimd.collective_compute(
                "AllGather",
                mybir.AluOpType.bypass,
                replica_groups=GROUPS,
                ins=[kslice_d[:]],
                outs=[kgath_d[:]],
            )
            for r in range(4):
                nc.sync.dma_start(
                    kT[:, :, r * QS : (r + 1) * QS], kgath_d[r]
                )

        def stage_v():
            vOwn = big.tile([P, 8, H * (HD + 1)], BF16, tag="vOwn")
            nc.any.memset(
                vOwn[:].rearrange("p k (h w) -> p k h w", w=HD + 1)[:, :, :, HD], 1.0
            )
            for kg in range(QS // 512):
                xt = stage_chunk(xv, kg * 512)
                for ktl in range(4):
                    kt = kg * 4 + ktl
                    ps = pp.tile([P, 512], F32, tag="s", name="psv")
                    for sub in range(4):
                        nc.tensor.matmul(
                            ps[:],
                            lhsT=xt[:, sub, ktl * P : (ktl + 1) * P],
                            rhs=wT["v"][:, sub, :],
                            start=(sub == 0),
                            stop=(sub == 3),
                        )
                    copy_cast(
                        vOwn[:, kt, :].rearrange("p (h w) -> p h w", w=HD + 1)[
                            :, :, :HD
                        ],
                        ps[:].rearrange("p (h d) -> p h d", d=HD),
                    )
            nc.sync.dma_start(vslice_d[:], vOwn[:])
            nc.gpsimd.collective_compute(
                "AllGather",
                mybir.AluOpType.bypass,
                replica_groups=GROUPS,
                ins=[vslice_d[:]],
                outs=[vgath_d[:]],
            )
            for r in range(4):
                nc.sync.dma_start(Vp[:, r * 8 : (r + 1) * 8, :], vgath_d[r])

        def stage_q():
            for qc in range(QS // 512):
                xt = stage_chunk(xq, qc * 512)
                for p4 in range(4):
                    ps = pp.tile([P, 512], F32, tag="s", name="psq")
                    for sub in range(4):
                        nc.tensor.matmul(
                            ps[:],
                            lhsT=wT["q"][:, sub, p4 * P : (p4 + 1) * P],
                            rhs=xt[:, sub, :],
                            start=(sub == 0),
                            stop=(sub == 3),
                        )
                    copy_cast(qT[:, p4, qc * 512 : (qc + 1) * 512], ps[:])

        stage_weights()
        stage_k()  # collective launches early, overlaps the rest of staging
        stage_v()
        stage_q()

        # ---- attention ----
        # Per (head-pair, 512-q chunk): stream key tiles. Each key tile kt:
        #   S^T psum [128, 2, 512] (head h in [:, h, :]), exp'd as one
        #   [128, 1024] instr alternating ScalarE/DVE into a bf16 at-tile,
        #   O^T accumulated in [65, 512] psum per head, lagging OLAG tiles.
        concatT = big.tile([P, 4, QS], BF16, tag="concatT")

        def fc_block(qt):
            ps = pp.tile([P, 512], F32, tag="fc", name="psf", bufs=1)
            for sub in range(4):
                nc.tensor.matmul(
                    ps[:],
                    lhsT=concatT[:, sub, qt * P : (qt + 1) * P],
                    rhs=wT["fc"][:, sub, :],
                    start=(sub == 0),
                    stop=(sub == 3),
                )
            ot = co.tile([P, 512], F32, tag="of", name="ot")
            copy_cast(ot[:], ps[:])
            nc.sync.dma_start(out[qt * P : (qt + 1) * P, :], ot[:])

        for qc in range(QS // 512):
            for p4 in range(4):
                po = [
                    ppo.tile([HD + 1, 512], F32, tag="o", name=f"po{_h}")
                    for _h in range(2)
                ]
                ats = {}

                def emit_S(kt, pss):
                    for h2 in range(2):
                        nc.tensor.matmul(
                            pss[:, h2, :],
                            lhsT=kT[
                                h2 * HD : (h2 + 1) * HD, p4, kt * P : (kt + 1) * P
                            ],
                            rhs=qT[
                                h2 * HD : (h2 + 1) * HD,
                                p4,
                                qc * 512 : (qc + 1) * 512,
                            ],
                            start=True,
                            stop=True,
                            tile_position=(h2 * HD, 0),
                        )

                def emit_O(kt):
                    at = ats.pop(kt)
                    for h2 in range(2):
                        h = p4 * 2 + h2
                        nc.tensor.matmul(
                            po[h2][:],
                            lhsT=Vp[:, kt, h * (HD + 1) : (h + 1) * (HD + 1)],
                            rhs=at[:, h2, :],
                            start=(kt == 0),
                            stop=(kt == KTT - 1),
                            skip_group_check=True,
                        )

                for kt in range(KTT + OLAG):
                    if kt < KTT:
                        pss = pp.tile([P, 2, 512], F32, tag="s", name="pss")
                        emit_S(kt, pss)
                        at = atp.tile([P, 2, 512], BF16, tag="at", name="at")
                        # 5:3 ACT:DVE split of the exp work, arranged so
                        # consecutive tail tiles alternate engines
                        if kt % 8 in (0, 2, 4, 5, 7):
                            nc.scalar.activation(at[:], pss[:], Exp, scale=SCALE)
                        else:
                            nc.vector.tensor_scalar(
                                at[:].bitcast(I16),
                                pss[:],
                                A_SCH,
                                B_SCH,
                                Mult,
                                Add,
                            )
                        ats[kt] = at
                    if kt >= OLAG:
                        emit_O(kt - OLAG)

                for h2 in range(2):
                    # reciprocal_approx_fast is SBUF-only (bitwise seed reads
                    # garbage from PSUM) -- stage the denominator row first.
                    dn = small.tile([1, 512], F32, tag="dn")
                    nc.vector.tensor_copy(dn[:], po[h2][HD : HD + 1, :])
                    rc = small.tile([1, 512], F32, tag="rc")
                    nc.vector.reciprocal_approx_fast(rc[:], dn[:])
                    rcb = small.tile([HD, 512], F32, tag="rcb")
                    nc.gpsimd.partition_broadcast(rcb[:], rc[:])
                    nc.vector.tensor_mul(
                        concatT[
                            h2 * HD : (h2 + 1) * HD, p4, qc * 512 : (qc + 1) * 512
                        ],
                        po[h2][:HD, :],
                        rcb[:],
                    )
            # this 512-q chunk's concatT is complete for all heads -> fc now
            for qt in range(qc * 4, (qc + 1) * 4):
                fc_block(qt)

    nc.compile()
    return nc


_NC_CACHE = None


def _get_nc():
    global _NC_CACHE
    if _NC_CACHE is None:
        _NC_CACHE = build_core_program()
    return _NC_CACHE


def make_in_maps(input_v, input_q, input_k, W_Q, W_K, W_V, W_fc):
    in_maps = []
    for c in range(NCORES):
        n, qlo = c // 4, (c % 4) * QS
        in_maps.append(
            {
                "xq": np.ascontiguousarray(input_q[n, qlo : qlo + QS]),
                "xk": np.ascontiguousarray(input_k[n, qlo : qlo + QS]),
                "xv": np.ascontiguousarray(input_v[n, qlo : qlo + QS]),
                "wq": W_Q,
                "wk": W_K,
                "wv": W_V,
                "wfc": W_fc,
            }
        )
    return in_maps


def assemble(results):
    out = np.empty((N_BATCH, S, E), np.float32)
    for c in range(NCORES):
        n, qlo = c // 4, (c % 4) * QS
        out[n, qlo : qlo + QS] = results[c]["out"]
    return out


def kernel(input_v, input_q, input_k, W_Q, W_K, W_V, W_fc):
    args = [
        np.asarray(a, dtype=np.float32)
        for a in (input_v, input_q, input_k, W_Q, W_K, W_V, W_fc)
    ]
    nc = _get_nc()
    res = run_bass_kernel_spmd(
        nc, make_in_maps(*args), core_ids=list(range(NCORES)), trace=False
    )
    return assemble(res.results)
